# revision 19
# baseline (speedup 1.0000x reference)
"""Trainium2 Bass kernel for nn_CustomModel_7378753814834.

Computation (see reference):
    d2[b, d]  = sum_k (x[b, k, d] - w[k, d])^2          (B=128, K=49, D=2048)
    kv[s,b,d] = exp(-d2[b,d] / (2 sigma_s^2))           (S=5 sigmas)
    out[s*B + b, k, d] = kv[s, b, d]    for all k       -> (640, 49, 2048) f32

Sharding: split D across the 8 cores (DL = 256 per core). Each core gets
x[:, :, c*DL:(c+1)*DL] transposed on host to d-major [b, d', k] (so the
k-reduction is over the contiguous innermost axis), computes d2 for all 128
batches with batch on the SBUF partition axis, and writes its D-slice of the
full sigma-major output (640, 49, DL). Host concatenates along d.

Pipeline per core:
  - w slice loaded once (50KB) into one partition; broadcast to all 128
    partitions on GpSimd in d'-chunks (no 6.4MB HBM broadcast read).
  - x loaded in NCH d'-chunks; per chunk: DVE sub -> ACT square -> DVE
    contiguous reduce over k into D2[:, chunk].
  - per sigma: ACT exp(-inv*d2), DVE-build KV7 (kv replicated 7x along k)
    and a single DMA per sigma writes kv broadcast over k (7 x 7KB
    descriptors per partition), alternating between the two HWDGE rings.
"""

import numpy as np

import concourse.bass as bass
import concourse.tile as tile
from concourse import bacc, mybir
from concourse import bass_utils

B, K, D = 128, 49, 2048
NCORES = 8
DL = D // NCORES            # 256 d-columns per core
F = K * DL                  # 12544 free elements per partition
S = 5
SIGMAS = [1.0, 2.0, 3.0, 4.0, 5.0]
INVS = [1.0 / (2.0 * s * s) for s in SIGMAS]
K7 = 7                      # 49 = 7 x 7

NCH = 8                     # d'-chunks for the load/compute pipeline
DC = DL // NCH              # 32 d'-columns per chunk

FP32 = mybir.dt.float32

# Knobs for experimentation from test.py
TRACE = False
TRACE_DIR = None
LAST = None          # last BassKernelResults (exec_time_ns when TRACE)

_compiled = None     # cached compiled Bass module


def _build_kernel():
    from contextlib import ExitStack

    nc = bacc.Bacc(
        "TRN2",
        target_bir_lowering=False,
        debug=False,
        enable_asserts=False,
        num_devices=NCORES,
    )
    # x is d-major: [b, d', k] flattened to [B, DL*K]
    x = nc.dram_tensor("x", [B, F], FP32, kind="ExternalInput")
    # w is d-major, single copy; broadcast across partitions on-chip
    w = nc.dram_tensor("w", [1, F], FP32, kind="ExternalInput")
    # out keeps the graded layout: [(s b), k, d']
    out = nc.dram_tensor("out", [S * B, F], FP32, kind="ExternalOutput")

    with tile.TileContext(nc) as tc, ExitStack() as ctx:
        const = ctx.enter_context(tc.tile_pool(name="const", bufs=1))
        xin = ctx.enter_context(tc.tile_pool(name="xin", bufs=4))
        work = ctx.enter_context(tc.tile_pool(name="work", bufs=4))
        kvp = ctx.enter_context(tc.tile_pool(name="kvp", bufs=5))

        W1 = const.tile([1, F], FP32)
        nc.sync.dma_start(W1[:], w.ap())
        D2 = const.tile([B, DL], FP32)

        x_v = x.ap().rearrange("b (d k) -> b d k", k=K)
        wpool = ctx.enter_context(tc.tile_pool(name="wpool", bufs=3))
        for c in range(NCH):
            sl = slice(c * DC * K, (c + 1) * DC * K)
            dsl = slice(c * DC, (c + 1) * DC)
            # broadcast this w chunk to all partitions (GpSimd, ~3.3us) while
            # the x chunk streams in on the sync HWDGE ring
            Wc = wpool.tile([B, DC * K], FP32, tag="wc")
            nc.gpsimd.partition_broadcast(Wc[:], W1[0:1, sl], channels=B)
            Xc = xin.tile([B, DC * K], FP32, tag="xc")
            nc.sync.dma_start(Xc[:], x_v[:, dsl, :])

            DIFF = work.tile([B, DC * K], FP32, tag="diff")
            nc.vector.tensor_sub(DIFF[:], Xc[:], Wc[:])
            SQ = work.tile([B, DC * K], FP32, tag="sq")
            nc.scalar.square(SQ[:], DIFF[:])
            nc.vector.tensor_reduce(
                out=D2[:, c * DC : (c + 1) * DC],
                in_=SQ[:].rearrange("b (d k) -> b d k", k=K),
                axis=mybir.AxisListType.X,
                op=mybir.AluOpType.add,
            )

        out_v = out.ap().rearrange("(s b) (ko f) -> s b ko f", s=S, ko=K7)
        for s in range(S):
            KV = kvp.tile([B, DL], FP32, tag="kv")
            nc.scalar.activation(
                KV[:], D2[:], mybir.ActivationFunctionType.Exp, scale=-INVS[s]
            )
            # replicate kv 7x along k (inner k7), f = [k7i, d'] = 7*256
            KV7 = kvp.tile([B, K7 * DL], FP32, tag="kv7")
            cp_eng = nc.vector if s % 2 == 0 else nc.gpsimd
            cp_eng.tensor_copy(
                KV7[:].rearrange("b (j d) -> b j d", j=K7),
                KV[:].unsqueeze(1).broadcast_to([B, K7, DL]),
            )
            # one DMA per sigma: outer k7 broadcast of the 7KB KV7 line.
            # Must stay a full-128-partition DMA: partial partition ranges
            # are dealt to fewer engines AND scramble the engine<->SBUF-port
            # pairing, halving the per-engine rate.
            src = KV7[:].unsqueeze(1).broadcast_to([B, K7, K7 * DL])
            eng = nc.sync if s % 2 == 0 else nc.scalar
            eng.dma_start(out_v[s], src)

    nc.compile()
    return nc


def _get_compiled():
    global _compiled
    if _compiled is None:
        _compiled = _build_kernel()
    return _compiled


def kernel(x, weight):
    x = np.asarray(x, dtype=np.float32)
    weight = np.asarray(weight, dtype=np.float32)
    assert x.shape == (B, K, D) and weight.shape == (1, K, D)

    nc = _get_compiled()

    in_maps = []
    for c in range(NCORES):
        # d-major per-core slices: [b, d', k]
        xs = np.ascontiguousarray(
            x[:, :, c * DL : (c + 1) * DL].transpose(0, 2, 1)
        ).reshape(B, F)
        ws = np.ascontiguousarray(
            weight[0, :, c * DL : (c + 1) * DL].T
        ).reshape(1, F)
        in_maps.append({"x": xs, "w": ws})

    res = bass_utils.run_bass_kernel_spmd(
        nc,
        in_maps,
        core_ids=list(range(NCORES)),
        trace=TRACE,
        tmpdir=TRACE_DIR,
    )
    global LAST
    LAST = res

    out = np.empty((S * B, K, D), dtype=np.float32)
    for c in range(NCORES):
        out[:, :, c * DL : (c + 1) * DL] = res.results[c]["out"].reshape(S * B, K, DL)
    return out


# revision 21
# speedup vs baseline: 1.1551x; 1.1551x over previous
"""Trainium2 Bass kernel for nn_CustomModel_7378753814834.

Computation (see reference):
    d2[b, d]  = sum_k (x[b, k, d] - w[k, d])^2          (B=128, K=49, D=2048)
    kv[s,b,d] = exp(-d2[b,d] / (2 sigma_s^2))           (S=5 sigmas)
    out[s*B + b, k, d] = kv[s, b, d]    for all k       -> (640, 49, 2048) f32

Sharding: split D across the 8 cores (DL = 256 per core). Each core gets
x[:, :, c*DL:(c+1)*DL] transposed on host to d-major [b, d', k] (so the
k-reduction is over the contiguous innermost axis), computes d2 for all 128
batches with batch on the SBUF partition axis, and writes its D-slice of the
full sigma-major output (640, 49, DL). Host concatenates along d.

Schedule per core:
  - All loads ride the sync HWDGE ring, interleaved (w quarter, then its two
    x chunks) so compute starts ~12us in; the scalar ring stays free.
  - Per d'-chunk: DVE sub -> ACT square -> DVE reduce over contiguous k.
  - d2 completes in two d'-halves. As soon as half 0 is done, sigma 0/1
    writes for that half start (512B-run descriptors), overlapping the rest
    of phase A; sigmas 2-4 are written as single full-d' DMAs (7KB runs).
  - All output DMAs use full-128-partition sources: partial partition
    ranges are dealt to fewer DMA engines and scramble the engine<->port
    pairing, halving throughput.
"""

import numpy as np

import concourse.bass as bass
import concourse.tile as tile
from concourse import bacc, mybir
from concourse import bass_utils

B, K, D = 128, 49, 2048
NCORES = 8
DL = D // NCORES            # 256 d-columns per core
F = K * DL                  # 12544 free elements per partition
S = 5
SIGMAS = [1.0, 2.0, 3.0, 4.0, 5.0]
INVS = [1.0 / (2.0 * s * s) for s in SIGMAS]
K7 = 7                      # 49 = 7 x 7
DH = DL // 2                # 128: d'-half

NCH = 8                     # d'-chunks for the load/compute pipeline
DC = DL // NCH              # 32 d'-columns per chunk

FP32 = mybir.dt.float32

# Knobs for experimentation from test.py
TRACE = False
TRACE_DIR = None
LAST = None          # last BassKernelResults (exec_time_ns when TRACE)

_compiled = None     # cached compiled Bass module

NSPLIT = 2           # sigmas written as two d'-half DMAs (early start)


def _build_kernel():
    from contextlib import ExitStack

    nc = bacc.Bacc(
        "TRN2",
        target_bir_lowering=False,
        debug=False,
        enable_asserts=False,
        num_devices=NCORES,
    )
    # x is d-major: [b, d', k] flattened to [B, DL*K]
    x = nc.dram_tensor("x", [B, F], FP32, kind="ExternalInput")
    # w is d-major and host-replicated across all 128 partitions
    w = nc.dram_tensor("w", [B, F], FP32, kind="ExternalInput")
    # out keeps the graded layout: [(s b), k, d']
    out = nc.dram_tensor("out", [S * B, F], FP32, kind="ExternalOutput")

    with tile.TileContext(nc) as tc, ExitStack() as ctx:
        const = ctx.enter_context(tc.tile_pool(name="const", bufs=1))
        xin = ctx.enter_context(tc.tile_pool(name="xin", bufs=4))
        work = ctx.enter_context(tc.tile_pool(name="work", bufs=4))
        kvp = ctx.enter_context(tc.tile_pool(name="kvp", bufs=5))

        WB = const.tile([B, F], FP32)
        D2 = const.tile([B, DL], FP32)

        x_v = x.ap().rearrange("b (d k) -> b d k", k=K)
        w_v = w.ap().rearrange("b (d k) -> b d k", k=K)

        # loads interleaved on the sync ring: w quarter q covers chunks
        # 2q, 2q+1 and is emitted just before them (ring FIFO = arrival order)
        for q in range(4):
            qd = slice(q * 2 * DC, (q + 1) * 2 * DC)
            nc.sync.dma_start(WB[:, q * 2 * DC * K : (q + 1) * 2 * DC * K],
                              w_v[:, qd, :])
            for i in range(2):
                c = 2 * q + i
                sl = slice(c * DC * K, (c + 1) * DC * K)
                dsl = slice(c * DC, (c + 1) * DC)
                Xc = xin.tile([B, DC * K], FP32, tag="xc")
                nc.sync.dma_start(Xc[:], x_v[:, dsl, :])

                DIFF = work.tile([B, DC * K], FP32, tag="diff")
                nc.vector.tensor_sub(DIFF[:], Xc[:], WB[:, sl])
                SQ = work.tile([B, DC * K], FP32, tag="sq")
                nc.scalar.square(SQ[:], DIFF[:])
                nc.vector.tensor_reduce(
                    out=D2[:, c * DC : (c + 1) * DC],
                    in_=SQ[:].rearrange("b (d k) -> b d k", k=K),
                    axis=mybir.AxisListType.X,
                    op=mybir.AluOpType.add,
                )

        out_v = out.ap().rearrange("(s b) (ko ki d) -> s b ko ki d",
                                   s=S, ko=K7, ki=K7)
        KVs = []
        KV7s = []
        for s in range(S):
            KVs.append(kvp.tile([B, DL], FP32, tag="kv", name=f"kv{s}"))
            KV7s.append(
                kvp.tile([B, K7 * DL], FP32, tag="kv7", name=f"kv7_{s}")
            )
        ring = [nc.sync, nc.scalar]
        ri = 0
        for h in range(2):
            hd = slice(h * DH, (h + 1) * DH)
            for s in range(S):
                KV, KV7 = KVs[s], KV7s[s]
                nc.scalar.activation(
                    KV[:, hd], D2[:, hd],
                    mybir.ActivationFunctionType.Exp, scale=-INVS[s],
                )
                kv7v = KV7[:].rearrange("b (j d) -> b j d", j=K7)
                # h0 builds on GpSimd (off the DVE critical path), h1 on DVE
                cp_eng = nc.gpsimd if h == 0 else nc.vector
                cp_eng.tensor_copy(
                    kv7v[:, :, hd],
                    KV[:, hd].unsqueeze(1).broadcast_to([B, K7, DH]),
                )
                if s < NSPLIT:
                    # early half-writes for the first sigmas; one DMA per
                    # outer k-group to keep APs <= 3 dims
                    for j in range(K7):
                        ring[ri % 2].dma_start(
                            out_v[s][:, j, :, hd], kv7v[:, :, hd]
                        )
                        ri += 1
                elif h == 1:
                    # single full-d' write (7KB descriptor runs)
                    src = KV7[:].unsqueeze(1).broadcast_to([B, K7, K7 * DL])
                    ring[ri % 2].dma_start(
                        out_v[s].rearrange("b ko ki d -> b ko (ki d)"), src
                    )
                    ri += 1

    nc.compile()
    return nc


def _get_compiled():
    global _compiled
    if _compiled is None:
        _compiled = _build_kernel()
    return _compiled


def kernel(x, weight):
    x = np.asarray(x, dtype=np.float32)
    weight = np.asarray(weight, dtype=np.float32)
    assert x.shape == (B, K, D) and weight.shape == (1, K, D)

    nc = _get_compiled()

    in_maps = []
    for c in range(NCORES):
        # d-major per-core slices: [b, d', k]
        xs = np.ascontiguousarray(
            x[:, :, c * DL : (c + 1) * DL].transpose(0, 2, 1)
        ).reshape(B, F)
        ws = np.ascontiguousarray(
            np.broadcast_to(
                weight[0, :, c * DL : (c + 1) * DL].T.reshape(1, F), (B, F)
            )
        )
        in_maps.append({"x": xs, "w": ws})

    res = bass_utils.run_bass_kernel_spmd(
        nc,
        in_maps,
        core_ids=list(range(NCORES)),
        trace=TRACE,
        tmpdir=TRACE_DIR,
    )
    global LAST
    LAST = res

    out = np.empty((S * B, K, D), dtype=np.float32)
    for c in range(NCORES):
        out[:, :, c * DL : (c + 1) * DL] = res.results[c]["out"].reshape(S * B, K, DL)
    return out
